# revision 15
# baseline (speedup 1.0000x reference)
"""Masked-gather L1 loss on 8 Trainium2 NeuronCores.

Strategy (data-parallel over batch, 4 batches per core):
  - HBM-roofline bound: each core must stream its 4 pred batches
    (4 x 13.1 MB f32) from HBM; everything else hides under that.
  - Host: sort each batch's 1024 indices, split into 5 quantile groups
    with static column windows (caps are x32 for the gather ucode,
    window starts 64B-aligned); permute target/mask to match. The
    capacities sum to exactly K, so no padding for uniform inputs.
  - Device: stream pred[b] in 5 column chunks; as soon as a group's
    window is resident, GPSIMD ap_gather pulls its columns, then
    DVE/ACT/PE compute |mid - tgt| -> ones^T matmul -> mask -> scalar.
    Gather ucode (~28ns/idx, ~122us/core total) pipelines against the
    ~130-160us pred stream; only the last 64-index gather + a ~6us
    compute/exit tail are exposed.
  - Each core returns [sum_b sum_ck m_k|t-p|, sum_b sum_k m_k]; host
    combines the 8 partial pairs and applies total / (mask_sum*C + eps).
  - If a (non-default) input's indices don't fit the static windows,
    fall back to the original single-gather-per-batch kernel.
"""

import sys

sys.path.insert(0, "/opt/trn_rl_repo")

import numpy as np

B, C, H, W = 32, 128, 160, 160
K = 1024
HW = H * W
N_CORES = 8
BPC = B // N_CORES  # batches per core
EPS = 1e-5

# Static gather-group windows (columns of pred) and capacities.
# Chosen so uniform indices sorted into quantile groups fit with
# margin; greedy assignment below handles small spills. The last
# groups are small so the exposed gather tail after the final DMA
# chunk lands is minimal.
# capacities must be multiples of 32 (gather ucode granularity)
GROUP_CAPS = (288, 256, 256, 160, 64)  # sum = 1024 = KP (zero padding)
# window starts must be 16-element (64 B) aligned for ap_gather's SBUF base
GROUP_LO = (0, 5200, 11584, 18304, 22704)
GROUP_HI = (8400, 15000, 21200, 24600, 25600)
CHUNKS = (0, 8400, 15000, 21200, 24600, 25600)  # pred DMA chunk boundaries
NG = len(GROUP_CAPS)
KP = sum(GROUP_CAPS)
SLOTS = KP // 16  # 68 idx slots per partition per batch

_CACHE = {}


def _build_fast():
    from contextlib import ExitStack

    from concourse import bacc, mybir, tile

    f32 = mybir.dt.float32
    i16 = mybir.dt.int16

    nc = bacc.Bacc(
        "TRN2",
        target_bir_lowering=False,
        debug=False,
        num_devices=N_CORES,
        dynamic_dma_scratch_size=4096,
    )

    pred_d = nc.dram_tensor("pred", [BPC, C, HW], f32, kind="ExternalInput")
    tgt_d = nc.dram_tensor("tgt", [BPC, C, KP], f32, kind="ExternalInput")
    idx_d = nc.dram_tensor("idx", [C, BPC * SLOTS], i16, kind="ExternalInput")
    msk_d = nc.dram_tensor("msk", [BPC, KP], f32, kind="ExternalInput")
    out_d = nc.dram_tensor("out", [1, 2], f32, kind="ExternalOutput")

    with tile.TileContext(nc) as tc, ExitStack() as ctx:
        pred_pool = ctx.enter_context(tc.tile_pool(name="pred", bufs=2))
        tgt_pool = ctx.enter_context(tc.tile_pool(name="tgt", bufs=2))
        msk_pool = ctx.enter_context(tc.tile_pool(name="msk", bufs=1))
        gt_pool = ctx.enter_context(tc.tile_pool(name="gt", bufs=4))
        singles = ctx.enter_context(tc.tile_pool(name="singles", bufs=1))
        psum = ctx.enter_context(tc.tile_pool(name="psum", bufs=6, space="PSUM"))

        idx_t = singles.tile([C, BPC * SLOTS], i16)
        nc.scalar.dma_start(idx_t[:], idx_d.ap()[:])
        ones_t = singles.tile([C, 1], f32)
        nc.vector.memset(ones_t[:], 1.0)
        acc_t = singles.tile([1, 2 * BPC], f32)
        nc.vector.memset(acc_t[:], 0.0)
        tmp_t = singles.tile([1, NG * BPC], f32)
        msum_t = singles.tile([1, BPC], f32)
        fin_t = singles.tile([1, 2], f32)

        for b in range(BPC):
            pt = pred_pool.tile([C, HW], f32)
            for c in range(NG):
                lo, hi = CHUNKS[c], CHUNKS[c + 1]
                nc.sync.dma_start(pt[:, lo:hi], pred_d.ap()[b, :, lo:hi])
            tt = tgt_pool.tile([C, KP], f32)
            nc.scalar.dma_start(tt[:], tgt_d.ap()[b])
            mt = msk_pool.tile([1, KP], f32)
            nc.scalar.dma_start(mt[:], msk_d.ap()[b : b + 1])

            off = 0
            for g in range(NG):
                cap = GROUP_CAPS[g]
                lo, hi = GROUP_LO[g], GROUP_HI[g]
                gt = gt_pool.tile([C, cap], f32)
                nc.gpsimd.ap_gather(
                    gt[:],
                    pt[:, lo:hi],
                    idx_t[:, b * SLOTS + off // 16 : b * SLOTS + (off + cap) // 16],
                    channels=C,
                    num_elems=hi - lo,
                    d=1,
                    num_idxs=cap,
                )
                nc.vector.tensor_tensor(
                    gt[:], gt[:], tt[:, off : off + cap], op=mybir.AluOpType.subtract
                )
                nc.scalar.activation(
                    gt[:], gt[:], mybir.ActivationFunctionType.Abs
                )
                ps = psum.tile([1, cap], f32)
                nc.tensor.matmul(ps[:], ones_t[:], gt[:])
                nc.vector.tensor_tensor(
                    ps[:], ps[:], mt[:, off : off + cap], op=mybir.AluOpType.mult
                )
                t = tmp_t[:, NG * b + g : NG * b + g + 1]
                nc.vector.tensor_reduce(
                    t, ps[:], axis=mybir.AxisListType.X, op=mybir.AluOpType.add
                )
                nc.vector.tensor_tensor(
                    acc_t[:, b : b + 1], acc_t[:, b : b + 1], t,
                    op=mybir.AluOpType.add,
                )
                off += cap

            nc.vector.tensor_reduce(
                msum_t[:, b : b + 1],
                mt[:],
                axis=mybir.AxisListType.X,
                op=mybir.AluOpType.add,
            )
            nc.vector.tensor_tensor(
                acc_t[:, BPC + b : BPC + b + 1],
                acc_t[:, BPC + b : BPC + b + 1],
                msum_t[:, b : b + 1],
                op=mybir.AluOpType.add,
            )

        nc.vector.tensor_reduce(
            fin_t[:, 0:1],
            acc_t[:, 0:BPC],
            axis=mybir.AxisListType.X,
            op=mybir.AluOpType.add,
        )
        nc.vector.tensor_reduce(
            fin_t[:, 1:2],
            acc_t[:, BPC : 2 * BPC],
            axis=mybir.AxisListType.X,
            op=mybir.AluOpType.add,
        )
        nc.sync.dma_start(out_d.ap()[:], fin_t[:])

    nc.compile()
    return nc


def _build_fallback():
    """Original kernel: one full-batch gather per batch (no sorting needed)."""
    from contextlib import ExitStack

    from concourse import bacc, mybir, tile

    f32 = mybir.dt.float32
    i16 = mybir.dt.int16

    nc = bacc.Bacc(
        "TRN2",
        target_bir_lowering=False,
        debug=False,
        num_devices=N_CORES,
        dynamic_dma_scratch_size=4096,
    )

    pred_d = nc.dram_tensor("pred", [BPC, C, HW], f32, kind="ExternalInput")
    target_d = nc.dram_tensor("target", [BPC, C, K], f32, kind="ExternalInput")
    idx_d = nc.dram_tensor("idx", [C, BPC * (K // 16)], i16, kind="ExternalInput")
    mask_d = nc.dram_tensor("mask", [BPC, K], f32, kind="ExternalInput")
    out_d = nc.dram_tensor("out", [1, 2], f32, kind="ExternalOutput")

    IDXW = K // 16

    with tile.TileContext(nc) as tc, ExitStack() as ctx:
        pred_pool = ctx.enter_context(tc.tile_pool(name="pred", bufs=2))
        mid_pool = ctx.enter_context(tc.tile_pool(name="mid", bufs=2))
        tgt_pool = ctx.enter_context(tc.tile_pool(name="tgt", bufs=1))
        msk_pool = ctx.enter_context(tc.tile_pool(name="msk", bufs=1))
        singles = ctx.enter_context(tc.tile_pool(name="singles", bufs=1))
        psum = ctx.enter_context(tc.tile_pool(name="psum", bufs=2, space="PSUM"))

        idx_t = singles.tile([C, BPC * IDXW], i16)
        nc.sync.dma_start(idx_t[:], idx_d.ap()[:])
        ones_t = singles.tile([C, 1], f32)
        nc.vector.memset(ones_t[:], 1.0)
        acc_t = singles.tile([1, 2 * BPC], f32)
        nc.vector.memset(acc_t[:], 0.0)
        tmp_t = singles.tile([1, 2], f32)
        fin_t = singles.tile([1, 2], f32)

        for b in range(BPC):
            pt = pred_pool.tile([C, HW], f32)
            nc.sync.dma_start(pt[:], pred_d.ap()[b])
            tt = tgt_pool.tile([C, K], f32)
            nc.sync.dma_start(tt[:], target_d.ap()[b])
            mt = msk_pool.tile([1, K], f32)
            nc.sync.dma_start(mt[:], mask_d.ap()[b : b + 1])

            gt = mid_pool.tile([C, K], f32)
            nc.gpsimd.ap_gather(
                gt[:],
                pt[:],
                idx_t[:, b * IDXW : (b + 1) * IDXW],
                channels=C,
                num_elems=HW,
                d=1,
                num_idxs=K,
            )
            nc.vector.tensor_tensor(
                gt[:], gt[:], tt[:], op=mybir.AluOpType.subtract
            )
            nc.scalar.activation(gt[:], gt[:], mybir.ActivationFunctionType.Abs)

            ps = psum.tile([1, K], f32)
            nc.tensor.matmul(ps[:, 0:512], ones_t[:], gt[:, 0:512])
            nc.tensor.matmul(ps[:, 512:1024], ones_t[:], gt[:, 512:1024])
            nc.vector.tensor_tensor(ps[:], ps[:], mt[:], op=mybir.AluOpType.mult)
            nc.vector.tensor_reduce(
                tmp_t[:, 0:1],
                ps[:],
                axis=mybir.AxisListType.X,
                op=mybir.AluOpType.add,
            )
            nc.vector.tensor_tensor(
                acc_t[:, b : b + 1],
                acc_t[:, b : b + 1],
                tmp_t[:, 0:1],
                op=mybir.AluOpType.add,
            )
            nc.vector.tensor_reduce(
                tmp_t[:, 1:2],
                mt[:],
                axis=mybir.AxisListType.X,
                op=mybir.AluOpType.add,
            )
            nc.vector.tensor_tensor(
                acc_t[:, BPC + b : BPC + b + 1],
                acc_t[:, BPC + b : BPC + b + 1],
                tmp_t[:, 1:2],
                op=mybir.AluOpType.add,
            )

        nc.vector.tensor_reduce(
            fin_t[:, 0:1],
            acc_t[:, 0:BPC],
            axis=mybir.AxisListType.X,
            op=mybir.AluOpType.add,
        )
        nc.vector.tensor_reduce(
            fin_t[:, 1:2],
            acc_t[:, BPC : 2 * BPC],
            axis=mybir.AxisListType.X,
            op=mybir.AluOpType.add,
        )
        nc.sync.dma_start(out_d.ap()[:], fin_t[:])

    nc.compile()
    return nc


def _get_nc(kind):
    if kind not in _CACHE:
        _CACHE[kind] = _build_fast() if kind == "fast" else _build_fallback()
    return _CACHE[kind]


def _wrap16(v):
    """ap_gather index layout: within each 16-partition group, index j lives
    at (partition j%16, slot j//16); replicated across the 8 groups."""
    n = v.shape[-1]
    return v.reshape(n // 16, 16).T  # [16, n//16]


def _assign_groups(idx_b):
    """Greedy assignment of one batch's sorted indices to the static groups.
    Returns (cols[KP], reb[KP] int16, valid[KP] bool) or None if infeasible."""
    order = np.argsort(idx_b, kind="stable")
    svals = idx_b[order]
    n_below = np.searchsorted(svals, GROUP_HI, side="left")
    cols = np.zeros(KP, dtype=np.int64)
    reb = np.zeros(KP, dtype=np.int16)
    valid = np.zeros(KP, dtype=bool)
    start = 0
    off = 0
    for g in range(NG):
        end = min(start + GROUP_CAPS[g], n_below[g])
        n_g = end - start
        if n_g > 0:
            if svals[start] < GROUP_LO[g]:
                return None
            cols[off : off + n_g] = order[start:end]
            reb[off : off + n_g] = (svals[start:end] - GROUP_LO[g]).astype(np.int16)
            valid[off : off + n_g] = True
        start = end
        off += GROUP_CAPS[g]
    if start != len(svals):
        return None
    return cols, reb, valid


def make_in_maps_fast(pred, target, indices, mask):
    pred = np.ascontiguousarray(np.asarray(pred), dtype=np.float32)
    target = np.ascontiguousarray(np.asarray(target), dtype=np.float32)
    indices = np.asarray(indices)
    mask = np.ascontiguousarray(np.asarray(mask), dtype=np.float32)

    predf = pred.reshape(B, C, HW)
    cols = np.zeros((B, KP), dtype=np.int64)
    reb = np.zeros((B, KP), dtype=np.int16)
    mperm = np.zeros((B, KP), dtype=np.float32)
    for b in range(B):
        r = _assign_groups(indices[b])
        if r is None:
            return None
        cols[b], reb[b], v = r
        mperm[b, v] = mask[b][cols[b, v]]
    tperm = np.take_along_axis(target, cols[:, None, :], axis=2)  # [B, C, KP]

    # wrap rebased indices: per batch [16, SLOTS], groups concatenated
    idxw = np.zeros((B, 16, SLOTS), dtype=np.int16)
    for b in range(B):
        off = 0
        for g in range(NG):
            cap = GROUP_CAPS[g]
            idxw[b, :, off // 16 : (off + cap) // 16] = _wrap16(
                reb[b, off : off + cap]
            )
            off += cap

    in_maps = []
    for core in range(N_CORES):
        sl = slice(core * BPC, (core + 1) * BPC)
        idx_core = np.tile(
            np.ascontiguousarray(idxw[sl].transpose(1, 0, 2)).reshape(
                16, BPC * SLOTS
            ),
            (C // 16, 1),
        )
        in_maps.append(
            {
                "pred": np.ascontiguousarray(predf[sl]),
                "tgt": tperm[sl],
                "idx": np.ascontiguousarray(idx_core),
                "msk": mperm[sl],
            }
        )
    return in_maps


def make_in_maps_fallback(pred, target, indices, mask):
    pred = np.ascontiguousarray(np.asarray(pred), dtype=np.float32)
    target = np.ascontiguousarray(np.asarray(target), dtype=np.float32)
    indices = np.asarray(indices)
    mask = np.ascontiguousarray(np.asarray(mask), dtype=np.float32)

    predf = pred.reshape(B, C, HW)
    idxw = indices.reshape(B, K // 16, 16).transpose(0, 2, 1)  # [B, 16, 64]
    idxt = np.tile(idxw, (1, C // 16, 1)).astype(np.int16)  # [B, 128, 64]

    in_maps = []
    for core in range(N_CORES):
        sl = slice(core * BPC, (core + 1) * BPC)
        idx_core = np.ascontiguousarray(
            idxt[sl].transpose(1, 0, 2)
        ).reshape(C, BPC * (K // 16))
        in_maps.append(
            {
                "pred": np.ascontiguousarray(predf[sl]),
                "target": target[sl],
                "idx": idx_core,
                "mask": mask[sl],
            }
        )
    return in_maps


def run(pred, target, indices, mask, trace=False, **rk_kwargs):
    from concourse.bass_utils import run_bass_kernel_spmd

    in_maps = make_in_maps_fast(pred, target, indices, mask)
    if in_maps is not None:
        nc = _get_nc("fast")
    else:
        nc = _get_nc("fallback")
        in_maps = make_in_maps_fallback(pred, target, indices, mask)
    res = run_bass_kernel_spmd(
        nc, in_maps, list(range(N_CORES)), trace=trace, **rk_kwargs
    )
    parts = np.stack([r["out"][0] for r in res.results])  # [8, 2]
    total = float(parts[:, 0].sum())
    mask_sum = float(parts[:, 1].sum())
    out = np.float32(total / (mask_sum * C + EPS))
    return out, res


def kernel(pred, target, indices, mask):
    out, _ = run(pred, target, indices, mask)
    return out
